# revision 21
# baseline (speedup 1.0000x reference)
"""Trainium2 Bass kernel for nn_Disp_61125974557155 (V2: segment-geometric exp).

Computes: trilinear upsample of a cost volume [B,1,48,64,128] ->
[B,193,256,512] (align_corners=False), softmin over disparity,
disparity regression -> [B,256,512].

Key idea: along d', the upsampled volume is piecewise-linear over 49
low-res segments (~4 d' each), so exp(-vol) is piecewise GEOMETRIC:
within segment s, e_j = A_s * q_s^j with A_s = exp(-(u_s + f0*Delta)),
q_s = exp(-c*Delta), c = 48/193. Instead of 193 exp rows per pixel the
scalar engine computes only ~194 rows per (t, r-pair) (A and q for two
H-phases), and the idle vector engine reconstructs the remaining rows
with chained bf16 multiplies (2x DVE mode). Edge segments (replicated
pads, Delta=0, 2 d's each) are folded into the j=0 stat weights; the
single 5-element segment gets its 5th row directly from the alpha
matmul (alpha + 4w column).

Per core (8 = 2 batches x 4 H'-quarters), t-major over 16 coarse rows:
  PE:  alpha/w matmul [100->100/94, 512] per (t, rp) (H-lerp folded),
       then 8 masked stat matmuls (M=4: S0/S1 x r-even/r-odd) per t,
       accumulated over layers j=0..3 in PSUM slots at partition 32*(t%4).
  ACT: one exp over [100, 4, 512] PSUM -> SBUF bf16 per t.
  DVE: W-lerp (bf16), 3 reconstruction TT muls per t, stat drains every
       4 t, partition-parallel finalize.
  DMA: stat scatter to pixel-major S0/S1, final divide, output.
"""

import numpy as np
from contextlib import ExitStack

import concourse.bass as bass
import concourse.bacc as bacc
import concourse.tile as tile
from concourse import mybir
from concourse.bass_utils import run_bass_kernel_spmd

F32 = mybir.dt.float32
BF16 = mybir.dt.bfloat16

MAXDISP = 192
DP = MAXDISP + 1
KD = 48
KP = KD + 2
NCORES = 8
C2 = KD / DP
WH = (0.625, 0.875, 0.125, 0.375)
NROW = 17
ROW_GROUPS = ((0, 1), (1, 1), (2, 2), (4, 4), (8, 4), (12, 4), (16, 1))


def _segments():
    segs = {}
    for dd in range(DP):
        i = (dd + 0.5) * C2 - 0.5
        fl = int(np.floor(i))
        segs.setdefault(fl, []).append((dd, i - fl))
    int_fls = [fl for fl in sorted(segs) if 0 <= fl <= 46]
    edge_fls = [-1, 47]
    five = [fl for fl in int_fls if len(segs[fl]) == 5]
    assert len(int_fls) == 47 and len(five) == 1
    return segs, int_fls, edge_fls, five[0]


def _build_consts():
    segs, int_fls, edge_fls, five_fl = _segments()

    def hrow(r, k, wt):
        v = np.zeros(100)
        v[k] = (1 - WH[r]) * wt
        v[50 + k] = WH[r] * wt
        return v

    awA = np.zeros((2, 100, 100))
    awW = np.zeros((2, 100, 94))
    for rp in range(2):
        for ui, r in enumerate((2 * rp, 2 * rp + 1)):
            for s, fl in enumerate(int_fls):
                klo = fl + 1
                f0 = segs[fl][0][1]
                awA[rp][:, 47 * ui + s] = hrow(r, klo, 1 - f0) + hrow(r, klo + 1, f0)
                awW[rp][:, 47 * ui + s] = hrow(r, klo + 1, C2) + hrow(r, klo, -C2)
            for ei, fl in enumerate(edge_fls):
                klo = fl + 1
                f0 = segs[fl][0][1]
                awA[rp][:, 94 + 2 * ui + ei] = (
                    hrow(r, klo, 1 - f0) + hrow(r, klo + 1, f0)
                )
            klo = five_fl + 1
            f0 = segs[five_fl][0][1]
            a_col = hrow(r, klo, 1 - f0) + hrow(r, klo + 1, f0)
            w_col = hrow(r, klo + 1, C2) + hrow(r, klo, -C2)
            awA[rp][:, 98 + ui] = a_col + 4 * w_col

    # stat cols ordered (S0_r0, S1_r0, S0_r1, S1_r1): one DMA per (tau, rp)
    # scatters straight into the interleaved [j, (S0|S1), 512] S-matrix
    rmA = np.zeros((100, 4))
    rmE = np.zeros((3, 100, 4))
    for ui in range(2):
        for s, fl in enumerate(int_fls):
            ds = [t[0] for t in segs[fl]]
            rmA[47 * ui + s, 2 * ui] = 1
            rmA[47 * ui + s, 2 * ui + 1] = ds[0]
            for j in (1, 2, 3):
                rmE[j - 1][47 * ui + s, 2 * ui] = 1
                rmE[j - 1][47 * ui + s, 2 * ui + 1] = ds[j]
        for ei, fl in enumerate(edge_fls):
            ds = [t[0] for t in segs[fl]]
            rmA[94 + 2 * ui + ei, 2 * ui] = 2
            rmA[94 + 2 * ui + ei, 2 * ui + 1] = ds[0] + ds[1]
        ds = [t[0] for t in segs[five_fl]]
        rmA[98 + ui, 2 * ui] = 1
        rmA[98 + ui, 2 * ui + 1] = ds[4]

    bf = mybir.dt.np(BF16)
    aw = np.concatenate(
        [awA[0], awW[0], awA[1], awW[1]], axis=1
    )  # [100, 388]
    rm = np.concatenate([rmA] + [rmE[j] for j in range(3)], axis=1)  # [100, 16]
    return aw.astype(bf), rm.astype(bf)


def _build_nc() -> bass.Bass:
    nc = bacc.Bacc()
    # xsdA/xsdB: host-duplicated, B shifted by one w-col, so every lerp
    # slice starts at an even element offset (keeps DVE 2x eligibility).
    xsdA = nc.declare_dram_parameter("xsdA", [100, NROW * 130], BF16, isOutput=False)
    xsdB = nc.declare_dram_parameter("xsdB", [100, NROW * 130], BF16, isOutput=False)
    aw = nc.declare_dram_parameter("aw", [100, 388], BF16, isOutput=False)
    rm = nc.declare_dram_parameter("rm", [100, 16], BF16, isOutput=False)
    outp = nc.declare_dram_parameter("out", [64, 512], F32, isOutput=True)

    xa_v = xsdA.rearrange("p (h w) -> p h w", h=NROW)
    xb_v = xsdB.rearrange("p (h w) -> p h w", h=NROW)
    aw_v = aw  # cols: rp0A(100) rp0W(94) rp1A(100) rp1W(94)

    mult = mybir.AluOpType.mult
    add = mybir.AluOpType.add
    exp_fn = mybir.ActivationFunctionType.Exp

    with ExitStack() as ctx:
        tc = ctx.enter_context(tile.TileContext(nc))
        singles = ctx.enter_context(tc.tile_pool(name="singles", bufs=1))
        tmp_pool = ctx.enter_context(tc.tile_pool(name="tmp", bufs=2))
        epool = ctx.enter_context(tc.tile_pool(name="epool", bufs=4))
        erp = ctx.enter_context(tc.tile_pool(name="erp", bufs=3))
        dpool = ctx.enter_context(tc.tile_pool(name="dpool", bufs=2))
        fin = ctx.enter_context(tc.tile_pool(name="fin", bufs=1))
        paw = ctx.enter_context(tc.tile_pool(name="paw", bufs=1, space="PSUM"))
        pstat = ctx.enter_context(tc.tile_pool(name="pstat", bufs=2, space="PSUM"))

        # ---- loads ----
        s_xa, s_xb = [], []
        for g, (g0, gn) in enumerate(ROW_GROUPS):
            t_xa = singles.tile([100, gn, 130], BF16, tag=f"xa{g}")
            nc.sync.dma_start(out=t_xa, in_=xa_v[:, g0 : g0 + gn, :])
            t_xb = singles.tile([100, gn, 130], BF16, tag=f"xb{g}")
            nc.scalar.dma_start(out=t_xb, in_=xb_v[:, g0 : g0 + gn, :])
            s_xa.append(t_xa)
            s_xb.append(t_xb)
        s_aw = {}
        off = 0
        for rp in range(2):
            t_a = singles.tile([100, 100], BF16, tag=f"awa{rp}")
            nc.gpsimd.dma_start(out=t_a, in_=aw_v[:, off : off + 100])
            off += 100
            t_w = singles.tile([100, 94], BF16, tag=f"aww{rp}")
            nc.gpsimd.dma_start(out=t_w, in_=aw_v[:, off : off + 94])
            off += 94
            s_aw[(rp, 0)] = t_a
            s_aw[(rp, 1)] = t_w
        s_rm = singles.tile([100, 4, 4], BF16, tag="rm")
        nc.gpsimd.dma_start(out=s_rm, in_=rm.rearrange("p (j c) -> p j c", j=4))

        # ---- W-lerp (bf16): rw0/1 = xsB[s] + c*(xsA[s]-xsB[s]);
        #      rw2/3 = xsA[s+2] + c*(xsB[s]-xsA[s+2])  (all even offsets) ----
        s_xsw = []
        for g, (g0, gn) in enumerate(ROW_GROUPS):
            t_w = singles.tile([100, gn, 4, 128], BF16, tag=f"xsw{g}")
            d0 = tmp_pool.tile([100, gn, 128], BF16, tag="d0")
            d1 = tmp_pool.tile([100, gn, 128], BF16, tag="d1")
            nc.vector.tensor_sub(d0, s_xa[g][:, :, 0:128], s_xb[g][:, :, 0:128])
            nc.vector.tensor_sub(d1, s_xb[g][:, :, 0:128], s_xa[g][:, :, 2:130])
            for rw, (coef, dt_, base) in enumerate(
                (
                    (0.375, 0, 0),
                    (0.125, 0, 0),
                    (0.875, 1, 2),
                    (0.625, 1, 2),
                )
            ):
                src_d = d0 if dt_ == 0 else d1
                src_x = (
                    s_xb[g][:, :, 0:128] if dt_ == 0 else s_xa[g][:, :, 2:130]
                )
                nc.vector.scalar_tensor_tensor(
                    out=t_w[:, :, rw, :],
                    in0=src_d,
                    scalar=coef,
                    in1=src_x,
                    op0=mult,
                    op1=add,
                )
            s_xsw.append(t_w)

        def xsw_row(l: int) -> bass.AP:
            for g, (g0, gn) in enumerate(ROW_GROUPS):
                if g0 <= l < g0 + gn:
                    return s_xsw[g][:, l - g0, :, :]
            raise IndexError(l)

        smat = fin.tile([64, 2, 512], F32, tag="smat")

        # ---- PE warmup: ~4.5us of back-to-back matmuls on the loaded
        # consts so the HAM clock gate releases (1.2 -> 2.4 GHz) before the
        # real work; results land in the paw tile and are overwritten ----
        pwarm = paw.tile([128, 4, 512], F32, tag="pw")
        for i in range(80):
            nc.tensor.matmul(
                pwarm[0:8, 0, 0:100],
                s_aw[(0, 0)][:, 0:8],
                s_aw[(0, 0)][:, 0:100],
                start=True,
                stop=True,
                skip_group_check=True,
            )

        # ---- main loop ----
        from collections import deque

        pending = deque()

        def emit_stats(ps, taq, et, tau, t0, is_block_end):
            for rp in range(2):
                for j in range(4):
                    if j == 0:
                        rhs = taq[0:100, 2 * rp, :]
                        lhsT = s_rm[0:100, 0, :]
                    else:
                        rhs = et[0:94, rp, j - 1, :]
                        lhsT = s_rm[0:94, j, :]
                    nc.tensor.matmul(
                        ps[32 * tau : 32 * tau + 4, rp, :],
                        lhsT,
                        rhs,
                        start=(j == 0),
                        stop=(j == 3),
                        skip_group_check=True,
                        tile_position=(0, 32 * tau),
                    )
            if is_block_end:
                sd = dpool.tile([128, 2, 512], F32, tag="sd")
                nc.vector.tensor_copy(sd[0:100, :, :], ps[0:100, :, :])
                for t2 in range(4):
                    j0 = 4 * (t0 + t2)
                    for rp in range(2):
                        eng = nc.sync if rp == 0 else nc.gpsimd
                        eng.dma_start(
                            out=smat[j0 + 2 * rp : j0 + 2 * rp + 2, :, :],
                            in_=sd[32 * t2 : 32 * t2 + 4, rp, :],
                        )

        ps = None
        for t in range(16):
            if t % 4 == 0:
                ps = pstat.tile([128, 2, 512], F32, tag="ps")
            tau = t % 4
            pw = paw.tile([128, 4, 512], F32, tag="pw")
            taq = epool.tile([128, 4, 512], BF16, tag="taq")
            for rp in range(2):
                rhs = xsw_row(t + rp).rearrange("p q s -> p (q s)")
                nc.tensor.matmul(
                    pw[0:100, 2 * rp, :], s_aw[(rp, 0)], rhs, start=True, stop=True
                )
                nc.tensor.matmul(
                    pw[0:94, 2 * rp + 1, :], s_aw[(rp, 1)], rhs, start=True, stop=True
                )
            nc.scalar.activation(
                taq[0:100, :, :], pw[0:100, :, :], exp_fn, scale=-1.0
            )
            # reconstruction: E_j = E_{j-1} * q  (bf16 2x, both rp per op)
            et = erp.tile([128, 2, 3, 512], BF16, tag="et")
            taq_v = taq.rearrange("p (rp aw) s -> p rp aw s", rp=2)
            nc.vector.tensor_mul(
                et[0:94, :, 0, :], taq_v[0:94, :, 0, :], taq_v[0:94, :, 1, :]
            )
            for j in (1, 2):
                nc.vector.tensor_mul(
                    et[0:94, :, j, :], et[0:94, :, j - 1, :], taq_v[0:94, :, 1, :]
                )
            pending.append((ps, taq, et, tau, t - tau, tau == 3))
            if len(pending) > 2:
                emit_stats(*pending.popleft())
        while pending:
            emit_stats(*pending.popleft())

        # ---- finalize: disp = S1 * recip(S0); un-permute (rw, s) -> w' ----
        rec = fin.tile([64, 512], F32, tag="rec")
        om = fin.tile([64, 512], F32, tag="om")
        nc.vector.reciprocal_approx_fast(out=rec, in_=smat[:, 0, :])
        nc.vector.tensor_mul(
            om.rearrange("j (s q) -> j q s", q=4),
            smat[:, 1, :].rearrange("j (q s) -> j q s", q=4),
            rec.rearrange("j (q s) -> j q s", q=4),
        )
        nc.sync.dma_start(out=outp[:, :], in_=om)

    nc.compile()
    return nc


_CACHE: dict = {}


def _shard_inputs(x: np.ndarray):
    """Edge-pad and slice per-core shards (layout + dtype cast only)."""
    xpad = np.pad(x[:, 0], ((0, 0), (1, 1), (1, 3), (1, 1)), mode="edge")
    aw, rm = _build_consts()
    bf = mybir.dt.np(BF16)
    in_maps = []
    for c in range(NCORES):
        b, q = divmod(c, 4)
        xs = xpad[b][:, 16 * q : 16 * q + 18, :]          # [50, 18, 130]
        xsd = np.concatenate([xs[:, 0:17, :], xs[:, 1:18, :]], axis=0)
        xsdA = np.ascontiguousarray(xsd.reshape(100, NROW * 130)).astype(bf)
        xsdB = np.zeros_like(xsd)
        xsdB[:, :, 0:129] = xsd[:, :, 1:130]
        xsdB = np.ascontiguousarray(xsdB.reshape(100, NROW * 130)).astype(bf)
        in_maps.append({"xsdA": xsdA, "xsdB": xsdB, "aw": aw, "rm": rm})
    return in_maps


def kernel(x: np.ndarray, _trace: bool = False, _tmpdir=None):
    x = np.asarray(x, dtype=np.float32)
    assert x.shape == (2, 1, 48, 64, 128), x.shape
    if "nc" not in _CACHE:
        _CACHE["nc"] = _build_nc()
    nc = _CACHE["nc"]
    in_maps = _shard_inputs(x)
    res = run_bass_kernel_spmd(
        nc, in_maps, list(range(NCORES)), trace=_trace, tmpdir=_tmpdir
    )
    out = np.zeros((2, 256, 512), dtype=np.float32)
    for c in range(NCORES):
        b, q = divmod(c, 4)
        out[b, 64 * q : 64 * (q + 1), :] = res.results[c]["out"]
    if _trace:
        return out, res
    return out


# revision 24
# speedup vs baseline: 1.2041x; 1.2041x over previous
"""Trainium2 Bass kernel for nn_Disp_61125974557155 (V2: segment-geometric exp).

Computes: trilinear upsample of a cost volume [B,1,48,64,128] ->
[B,193,256,512] (align_corners=False), softmin over disparity,
disparity regression -> [B,256,512].

Key idea: along d', the upsampled volume is piecewise-linear over 49
low-res segments (~4 d' each), so exp(-vol) is piecewise GEOMETRIC:
within segment s, e_j = A_s * q_s^j with A_s = exp(-(u_s + f0*Delta)),
q_s = exp(-c*Delta), c = 48/193. Instead of 193 exp rows per pixel the
scalar engine computes only ~194 rows per (t, r-pair) (A and q for two
H-phases), and the idle vector engine reconstructs the remaining rows
with chained bf16 multiplies (2x DVE mode). Edge segments (replicated
pads, Delta=0, 2 d's each) are folded into the j=0 stat weights; the
single 5-element segment gets its 5th row directly from the alpha
matmul (alpha + 4w column).

Per core (8 = 2 batches x 4 H'-quarters), t-major over 16 coarse rows:
  PE:  alpha/w matmul [100->100/94, 512] per (t, rp) (H-lerp folded),
       then 8 masked stat matmuls (M=4: S0/S1 x r-even/r-odd) per t,
       accumulated over layers j=0..3 in PSUM slots at partition 32*(t%4).
  ACT: one exp over [100, 4, 512] PSUM -> SBUF bf16 per t.
  DVE: W-lerp (bf16), 3 reconstruction TT muls per t, stat drains every
       4 t, partition-parallel finalize.
  DMA: stat scatter to pixel-major S0/S1, final divide, output.
"""

import numpy as np
from contextlib import ExitStack

import concourse.bass as bass
import concourse.bacc as bacc
import concourse.tile as tile
from concourse import mybir
from concourse.bass_utils import run_bass_kernel_spmd

F32 = mybir.dt.float32
BF16 = mybir.dt.bfloat16

MAXDISP = 192
DP = MAXDISP + 1
KD = 48
KP = KD + 2
NCORES = 8
C2 = KD / DP
WH = (0.625, 0.875, 0.125, 0.375)
NROW = 17
ROW_GROUPS = ((0, 1), (1, 1), (2, 2), (4, 4), (8, 4), (12, 4), (16, 1))


def _segments():
    segs = {}
    for dd in range(DP):
        i = (dd + 0.5) * C2 - 0.5
        fl = int(np.floor(i))
        segs.setdefault(fl, []).append((dd, i - fl))
    int_fls = [fl for fl in sorted(segs) if 0 <= fl <= 46]
    edge_fls = [-1, 47]
    five = [fl for fl in int_fls if len(segs[fl]) == 5]
    assert len(int_fls) == 47 and len(five) == 1
    return segs, int_fls, edge_fls, five[0]


def _build_consts():
    segs, int_fls, edge_fls, five_fl = _segments()

    def hrow(r, k, wt):
        v = np.zeros(100)
        v[k] = (1 - WH[r]) * wt
        v[50 + k] = WH[r] * wt
        return v

    awA = np.zeros((2, 100, 100))
    awW = np.zeros((2, 100, 94))
    for rp in range(2):
        for ui, r in enumerate((2 * rp, 2 * rp + 1)):
            for s, fl in enumerate(int_fls):
                klo = fl + 1
                f0 = segs[fl][0][1]
                awA[rp][:, 47 * ui + s] = hrow(r, klo, 1 - f0) + hrow(r, klo + 1, f0)
                awW[rp][:, 47 * ui + s] = hrow(r, klo + 1, C2) + hrow(r, klo, -C2)
            for ei, fl in enumerate(edge_fls):
                klo = fl + 1
                f0 = segs[fl][0][1]
                awA[rp][:, 94 + 2 * ui + ei] = (
                    hrow(r, klo, 1 - f0) + hrow(r, klo + 1, f0)
                )
            klo = five_fl + 1
            f0 = segs[five_fl][0][1]
            a_col = hrow(r, klo, 1 - f0) + hrow(r, klo + 1, f0)
            w_col = hrow(r, klo + 1, C2) + hrow(r, klo, -C2)
            awA[rp][:, 98 + ui] = a_col + 4 * w_col

    # stat cols ordered (S0_r0, S1_r0, S0_r1, S1_r1): one DMA per (tau, rp)
    # scatters straight into the interleaved [j, (S0|S1), 512] S-matrix
    rmA = np.zeros((100, 4))
    rmE = np.zeros((3, 100, 4))
    for ui in range(2):
        for s, fl in enumerate(int_fls):
            ds = [t[0] for t in segs[fl]]
            rmA[47 * ui + s, 2 * ui] = 1
            rmA[47 * ui + s, 2 * ui + 1] = ds[0]
            for j in (1, 2, 3):
                rmE[j - 1][47 * ui + s, 2 * ui] = 1
                rmE[j - 1][47 * ui + s, 2 * ui + 1] = ds[j]
        for ei, fl in enumerate(edge_fls):
            ds = [t[0] for t in segs[fl]]
            rmA[94 + 2 * ui + ei, 2 * ui] = 2
            rmA[94 + 2 * ui + ei, 2 * ui + 1] = ds[0] + ds[1]
        ds = [t[0] for t in segs[five_fl]]
        rmA[98 + ui, 2 * ui] = 1
        rmA[98 + ui, 2 * ui + 1] = ds[4]

    bf = mybir.dt.np(BF16)
    aw = np.concatenate(
        [awA[0], awW[0], awA[1], awW[1]], axis=1
    )  # [100, 388]
    rm = np.concatenate([rmA] + [rmE[j] for j in range(3)], axis=1)  # [100, 16]
    return aw.astype(bf), rm.astype(bf)


def _build_nc() -> bass.Bass:
    nc = bacc.Bacc()
    # xsdA/xsdB: host-duplicated, B shifted by one w-col, so every lerp
    # slice starts at an even element offset (keeps DVE 2x eligibility).
    xsdA = nc.declare_dram_parameter("xsdA", [100, NROW * 130], BF16, isOutput=False)
    xsdB = nc.declare_dram_parameter("xsdB", [100, NROW * 130], BF16, isOutput=False)
    aw = nc.declare_dram_parameter("aw", [100, 388], BF16, isOutput=False)
    rm = nc.declare_dram_parameter("rm", [100, 16], BF16, isOutput=False)
    outp = nc.declare_dram_parameter("out", [64, 512], F32, isOutput=True)

    xa_v = xsdA.rearrange("p (h w) -> p h w", h=NROW)
    xb_v = xsdB.rearrange("p (h w) -> p h w", h=NROW)
    aw_v = aw  # cols: rp0A(100) rp0W(94) rp1A(100) rp1W(94)

    mult = mybir.AluOpType.mult
    add = mybir.AluOpType.add
    exp_fn = mybir.ActivationFunctionType.Exp

    with ExitStack() as ctx:
        tc = ctx.enter_context(tile.TileContext(nc))
        singles = ctx.enter_context(tc.tile_pool(name="singles", bufs=1))
        tmp_pool = ctx.enter_context(tc.tile_pool(name="tmp", bufs=2))
        epool = ctx.enter_context(tc.tile_pool(name="epool", bufs=4))
        erp = ctx.enter_context(tc.tile_pool(name="erp", bufs=3))
        dpool = ctx.enter_context(tc.tile_pool(name="dpool", bufs=2))
        fin = ctx.enter_context(tc.tile_pool(name="fin", bufs=1))
        paw = ctx.enter_context(tc.tile_pool(name="paw", bufs=1, space="PSUM"))
        pstat = ctx.enter_context(tc.tile_pool(name="pstat", bufs=2, space="PSUM"))

        # ---- loads ----
        s_xa, s_xb = [], []
        for g, (g0, gn) in enumerate(ROW_GROUPS):
            t_xa = singles.tile([100, gn, 130], BF16, tag=f"xa{g}")
            nc.sync.dma_start(out=t_xa, in_=xa_v[:, g0 : g0 + gn, :])
            t_xb = singles.tile([100, gn, 130], BF16, tag=f"xb{g}")
            nc.scalar.dma_start(out=t_xb, in_=xb_v[:, g0 : g0 + gn, :])
            s_xa.append(t_xa)
            s_xb.append(t_xb)
        s_aw = {}
        off = 0
        for rp in range(2):
            t_a = singles.tile([100, 100], BF16, tag=f"awa{rp}")
            nc.gpsimd.dma_start(out=t_a, in_=aw_v[:, off : off + 100])
            off += 100
            t_w = singles.tile([100, 94], BF16, tag=f"aww{rp}")
            nc.gpsimd.dma_start(out=t_w, in_=aw_v[:, off : off + 94])
            off += 94
            s_aw[(rp, 0)] = t_a
            s_aw[(rp, 1)] = t_w
        s_rm = singles.tile([100, 4, 4], BF16, tag="rm")
        nc.gpsimd.dma_start(out=s_rm, in_=rm.rearrange("p (j c) -> p j c", j=4))

        # ---- W-lerp (bf16): rw0/1 = xsB[s] + c*(xsA[s]-xsB[s]);
        #      rw2/3 = xsA[s+2] + c*(xsB[s]-xsA[s+2])  (all even offsets) ----
        s_xsw = []
        for g, (g0, gn) in enumerate(ROW_GROUPS):
            t_w = singles.tile([100, gn, 4, 128], BF16, tag=f"xsw{g}")
            d0 = tmp_pool.tile([100, gn, 128], BF16, tag="d0")
            d1 = tmp_pool.tile([100, gn, 128], BF16, tag="d1")
            nc.vector.tensor_sub(d0, s_xa[g][:, :, 0:128], s_xb[g][:, :, 0:128])
            nc.vector.tensor_sub(d1, s_xb[g][:, :, 0:128], s_xa[g][:, :, 2:130])
            for rw, (coef, dt_, base) in enumerate(
                (
                    (0.375, 0, 0),
                    (0.125, 0, 0),
                    (0.875, 1, 2),
                    (0.625, 1, 2),
                )
            ):
                src_d = d0 if dt_ == 0 else d1
                src_x = (
                    s_xb[g][:, :, 0:128] if dt_ == 0 else s_xa[g][:, :, 2:130]
                )
                nc.vector.scalar_tensor_tensor(
                    out=t_w[:, :, rw, :],
                    in0=src_d,
                    scalar=coef,
                    in1=src_x,
                    op0=mult,
                    op1=add,
                )
            s_xsw.append(t_w)

        def xsw_row(l: int) -> bass.AP:
            for g, (g0, gn) in enumerate(ROW_GROUPS):
                if g0 <= l < g0 + gn:
                    return s_xsw[g][:, l - g0, :, :]
            raise IndexError(l)

        smat = fin.tile([64, 2, 512], F32, tag="smat")

        # ---- PE warmup: ~4.5us of back-to-back matmuls on the loaded
        # consts so the HAM clock gate releases (1.2 -> 2.4 GHz) before the
        # real work; results land in the paw tile and are overwritten ----
        pwarm = paw.tile([128, 4, 512], F32, tag="pw")
        for i in range(80):
            nc.tensor.matmul(
                pwarm[0:8, 0, 0:100],
                s_aw[(0, 0)][:, 0:8],
                s_aw[(0, 0)][:, 0:100],
                start=True,
                stop=True,
                skip_group_check=True,
            )

        # ---- main loop ----
        from collections import deque

        pending = deque()

        def emit_stats(ps, taq, et, tau, t0, is_block_end):
            # the two rp accumulation chains target different 32-aligned
            # partition bases (= array column groups), so their matmuls can
            # stream through the PE concurrently; j-major emission keeps
            # adjacent instructions conflict-free
            for j in range(4):
                for rp in range(2):
                    base = 32 * (2 * tau + rp)
                    if j == 0:
                        rhs = taq[0:100, 2 * rp, :]
                        lhsT = s_rm[0:100, 0, :]
                    else:
                        rhs = et[0:94, rp, j - 1, :]
                        lhsT = s_rm[0:94, j, :]
                    nc.tensor.matmul(
                        ps[base : base + 4, :],
                        lhsT,
                        rhs,
                        start=(j == 0),
                        stop=(j == 3),
                        skip_group_check=True,
                        tile_position=(0, base),
                    )
            if is_block_end:
                sd = dpool.tile([128, 512], F32, tag="sd")
                nc.vector.tensor_copy(sd[0:100, :], ps[0:100, :])
                for t2 in range(2):
                    for rp in range(2):
                        j0 = 4 * (t0 + t2) + 2 * rp
                        b = 32 * (2 * t2 + rp)
                        eng = nc.sync if rp == 0 else nc.gpsimd
                        eng.dma_start(
                            out=smat[j0 : j0 + 2, :, :],
                            in_=sd[b : b + 4, :],
                        )

        ps = None
        for t in range(16):
            if t % 2 == 0:
                ps = pstat.tile([128, 512], F32, tag="ps")
            tau = t % 2
            pw = paw.tile([128, 4, 512], F32, tag="pw")
            taq = epool.tile([128, 4, 512], BF16, tag="taq")
            for rp in range(2):
                rhs = xsw_row(t + rp).rearrange("p q s -> p (q s)")
                nc.tensor.matmul(
                    pw[0:100, 2 * rp, :], s_aw[(rp, 0)], rhs, start=True, stop=True
                )
                nc.tensor.matmul(
                    pw[0:94, 2 * rp + 1, :], s_aw[(rp, 1)], rhs, start=True, stop=True
                )
            nc.scalar.activation(
                taq[0:100, :, :], pw[0:100, :, :], exp_fn, scale=-1.0
            )
            # reconstruction: E_j = E_{j-1} * q  (bf16 2x, both rp per op)
            et = erp.tile([128, 2, 3, 512], BF16, tag="et")
            taq_v = taq.rearrange("p (rp aw) s -> p rp aw s", rp=2)
            nc.vector.tensor_mul(
                et[0:94, :, 0, :], taq_v[0:94, :, 0, :], taq_v[0:94, :, 1, :]
            )
            for j in (1, 2):
                nc.vector.tensor_mul(
                    et[0:94, :, j, :], et[0:94, :, j - 1, :], taq_v[0:94, :, 1, :]
                )
            pending.append((ps, taq, et, tau, t - tau, tau == 1))
            if len(pending) > 2:
                emit_stats(*pending.popleft())
        while pending:
            emit_stats(*pending.popleft())

        # ---- finalize: disp = S1 * recip(S0); un-permute (rw, s) -> w' ----
        rec = fin.tile([64, 512], F32, tag="rec")
        om = fin.tile([64, 512], F32, tag="om")
        nc.vector.reciprocal_approx_fast(out=rec, in_=smat[:, 0, :])
        nc.vector.tensor_mul(
            om.rearrange("j (s q) -> j q s", q=4),
            smat[:, 1, :].rearrange("j (q s) -> j q s", q=4),
            rec.rearrange("j (q s) -> j q s", q=4),
        )
        nc.sync.dma_start(out=outp[:, :], in_=om)

    nc.compile()
    return nc


_CACHE: dict = {}


def _shard_inputs(x: np.ndarray):
    """Edge-pad and slice per-core shards (layout + dtype cast only)."""
    xpad = np.pad(x[:, 0], ((0, 0), (1, 1), (1, 3), (1, 1)), mode="edge")
    aw, rm = _build_consts()
    bf = mybir.dt.np(BF16)
    in_maps = []
    for c in range(NCORES):
        b, q = divmod(c, 4)
        xs = xpad[b][:, 16 * q : 16 * q + 18, :]          # [50, 18, 130]
        xsd = np.concatenate([xs[:, 0:17, :], xs[:, 1:18, :]], axis=0)
        xsdA = np.ascontiguousarray(xsd.reshape(100, NROW * 130)).astype(bf)
        xsdB = np.zeros_like(xsd)
        xsdB[:, :, 0:129] = xsd[:, :, 1:130]
        xsdB = np.ascontiguousarray(xsdB.reshape(100, NROW * 130)).astype(bf)
        in_maps.append({"xsdA": xsdA, "xsdB": xsdB, "aw": aw, "rm": rm})
    return in_maps


def kernel(x: np.ndarray, _trace: bool = False, _tmpdir=None):
    x = np.asarray(x, dtype=np.float32)
    assert x.shape == (2, 1, 48, 64, 128), x.shape
    if "nc" not in _CACHE:
        _CACHE["nc"] = _build_nc()
    nc = _CACHE["nc"]
    in_maps = _shard_inputs(x)
    res = run_bass_kernel_spmd(
        nc, in_maps, list(range(NCORES)), trace=_trace, tmpdir=_tmpdir
    )
    out = np.zeros((2, 256, 512), dtype=np.float32)
    for c in range(NCORES):
        b, q = divmod(c, 4)
        out[b, 64 * q : 64 * (q + 1), :] = res.results[c]["out"]
    if _trace:
        return out, res
    return out


# revision 31
# speedup vs baseline: 1.2147x; 1.0088x over previous
"""Trainium2 Bass kernel for nn_Disp_61125974557155 (V2: segment-geometric exp).

Computes: trilinear upsample of a cost volume [B,1,48,64,128] ->
[B,193,256,512] (align_corners=False), softmin over disparity,
disparity regression -> [B,256,512].

Key idea: along d', the upsampled volume is piecewise-linear over 49
low-res segments (~4 d' each), so exp(-vol) is piecewise GEOMETRIC:
within segment s, e_j = A_s * q_s^j with A_s = exp(-(u_s + f0*Delta)),
q_s = exp(-c*Delta), c = 48/193. Instead of 193 exp rows per pixel the
scalar engine computes only ~194 rows per (t, r-pair) (A and q for two
H-phases), and the idle vector engine reconstructs the remaining rows
with chained bf16 multiplies (2x DVE mode). Edge segments (replicated
pads, Delta=0, 2 d's each) are folded into the j=0 stat weights; the
single 5-element segment gets its 5th row directly from the alpha
matmul (alpha + 4w column).

Per core (8 = 2 batches x 4 H'-quarters), t-major over 16 coarse rows:
  PE:  alpha/w matmul [100->100/94, 512] per (t, rp) (H-lerp folded),
       then 8 masked stat matmuls (M=4: S0/S1 x r-even/r-odd) per t,
       accumulated over layers j=0..3 in PSUM slots at partition 32*(t%4).
  ACT: one exp over [100, 4, 512] PSUM -> SBUF bf16 per t.
  DVE: W-lerp (bf16), 3 reconstruction TT muls per t, stat drains every
       4 t, partition-parallel finalize.
  DMA: stat scatter to pixel-major S0/S1, final divide, output.
"""

import numpy as np
from contextlib import ExitStack

import concourse.bass as bass
import concourse.bacc as bacc
import concourse.tile as tile
from concourse import mybir
from concourse.bass_utils import run_bass_kernel_spmd

F32 = mybir.dt.float32
BF16 = mybir.dt.bfloat16

MAXDISP = 192
DP = MAXDISP + 1
KD = 48
KP = KD + 2
NCORES = 8
C2 = KD / DP
WH = (0.625, 0.875, 0.125, 0.375)
NROW = 17
ROW_GROUPS = ((0, 1), (1, 1), (2, 2), (4, 4), (8, 4), (12, 4), (16, 1))


def _segments():
    segs = {}
    for dd in range(DP):
        i = (dd + 0.5) * C2 - 0.5
        fl = int(np.floor(i))
        segs.setdefault(fl, []).append((dd, i - fl))
    int_fls = [fl for fl in sorted(segs) if 0 <= fl <= 46]
    edge_fls = [-1, 47]
    five = [fl for fl in int_fls if len(segs[fl]) == 5]
    assert len(int_fls) == 47 and len(five) == 1
    return segs, int_fls, edge_fls, five[0]


def _build_consts():
    segs, int_fls, edge_fls, five_fl = _segments()

    def hrow(r, k, wt):
        v = np.zeros(100)
        v[k] = (1 - WH[r]) * wt
        v[50 + k] = WH[r] * wt
        return v

    # per (rp, r-parity ui): A-half [100, 50] = [47 interior | 2 edges |
    # 1 five-col], W-half [100, 47]. The two ui halves of each matmul run
    # at array column-group pairs {0,1} / {2,3} (partition bases 0 / 64).
    awA = np.zeros((2, 2, 100, 50))
    awW = np.zeros((2, 2, 100, 47))
    for rp in range(2):
        for ui, r in enumerate((2 * rp, 2 * rp + 1)):
            for s, fl in enumerate(int_fls):
                klo = fl + 1
                f0 = segs[fl][0][1]
                awA[rp][ui][:, s] = hrow(r, klo, 1 - f0) + hrow(r, klo + 1, f0)
                awW[rp][ui][:, s] = hrow(r, klo + 1, C2) + hrow(r, klo, -C2)
            for ei, fl in enumerate(edge_fls):
                klo = fl + 1
                f0 = segs[fl][0][1]
                awA[rp][ui][:, 47 + ei] = (
                    hrow(r, klo, 1 - f0) + hrow(r, klo + 1, f0)
                )
            klo = five_fl + 1
            f0 = segs[five_fl][0][1]
            a_col = hrow(r, klo, 1 - f0) + hrow(r, klo + 1, f0)
            w_col = hrow(r, klo + 1, C2) + hrow(r, klo, -C2)
            awA[rp][ui][:, 49] = a_col + 4 * w_col

    # stat cols ordered (S0_r0, S1_r0, S0_r1, S1_r1): one DMA per (tau, rp)
    # scatters straight into the interleaved [j, (S0|S1), 512] S-matrix.
    # Rows follow the split taq layout: ui-block at partition base 64*ui =
    # [47 interior | 2 edges | 1 five | 14 junk(zero-weight)].
    rmA = np.zeros((128, 4))
    rmE = np.zeros((3, 128, 4))
    for ui in range(2):
        base = 64 * ui
        for s, fl in enumerate(int_fls):
            ds = [t[0] for t in segs[fl]]
            rmA[base + s, 2 * ui] = 1
            rmA[base + s, 2 * ui + 1] = ds[0]
            for j in (1, 2, 3):
                rmE[j - 1][base + s, 2 * ui] = 1
                rmE[j - 1][base + s, 2 * ui + 1] = ds[j]
        for ei, fl in enumerate(edge_fls):
            ds = [t[0] for t in segs[fl]]
            rmA[base + 47 + ei, 2 * ui] = 2
            rmA[base + 47 + ei, 2 * ui + 1] = ds[0] + ds[1]
        ds = [t[0] for t in segs[five_fl]]
        rmA[base + 49, 2 * ui] = 1
        rmA[base + 49, 2 * ui + 1] = ds[4]

    bf = mybir.dt.np(BF16)
    aw = np.concatenate(
        [awA[0][0], awA[0][1], awW[0][0], awW[0][1],
         awA[1][0], awA[1][1], awW[1][0], awW[1][1]], axis=1
    )  # [100, 388]
    rm = np.concatenate([rmA] + [rmE[j] for j in range(3)], axis=1)  # [128, 16]
    return aw.astype(bf), rm.astype(bf)


def _build_nc() -> bass.Bass:
    nc = bacc.Bacc()
    # xsdA/xsdB: host-duplicated, B shifted by one w-col, so every lerp
    # slice starts at an even element offset (keeps DVE 2x eligibility).
    xsdA = nc.declare_dram_parameter("xsdA", [100, NROW * 130], BF16, isOutput=False)
    xsdB = nc.declare_dram_parameter("xsdB", [100, NROW * 130], BF16, isOutput=False)
    aw = nc.declare_dram_parameter("aw", [100, 388], BF16, isOutput=False)
    rm = nc.declare_dram_parameter("rm", [128, 16], BF16, isOutput=False)
    outp = nc.declare_dram_parameter("out", [64, 512], F32, isOutput=True)

    xa_v = xsdA.rearrange("p (h w) -> p h w", h=NROW)
    xb_v = xsdB.rearrange("p (h w) -> p h w", h=NROW)
    aw_v = aw  # cols: rp0A(100) rp0W(94) rp1A(100) rp1W(94)

    mult = mybir.AluOpType.mult
    add = mybir.AluOpType.add
    exp_fn = mybir.ActivationFunctionType.Exp

    with ExitStack() as ctx:
        tc = ctx.enter_context(tile.TileContext(nc))
        singles = ctx.enter_context(tc.tile_pool(name="singles", bufs=1))
        tmp_pool = ctx.enter_context(tc.tile_pool(name="tmp", bufs=2))
        epool = ctx.enter_context(tc.tile_pool(name="epool", bufs=4))
        erp = ctx.enter_context(tc.tile_pool(name="erp", bufs=3))
        dpool = ctx.enter_context(tc.tile_pool(name="dpool", bufs=2))
        fin = ctx.enter_context(tc.tile_pool(name="fin", bufs=1))
        paw = ctx.enter_context(tc.tile_pool(name="paw", bufs=1, space="PSUM"))
        pstat = ctx.enter_context(tc.tile_pool(name="pstat", bufs=2, space="PSUM"))

        # ---- loads ----
        s_xa, s_xb = [], []
        for g, (g0, gn) in enumerate(ROW_GROUPS):
            t_xa = singles.tile([100, gn, 130], BF16, tag=f"xa{g}")
            nc.sync.dma_start(out=t_xa, in_=xa_v[:, g0 : g0 + gn, :])
            t_xb = singles.tile([100, gn, 130], BF16, tag=f"xb{g}")
            nc.scalar.dma_start(out=t_xb, in_=xb_v[:, g0 : g0 + gn, :])
            s_xa.append(t_xa)
            s_xb.append(t_xb)
        s_aw = {}
        off = 0
        for rp in range(2):
            for kind, w in (("a0", 50), ("a1", 50), ("w0", 47), ("w1", 47)):
                t_c = singles.tile([100, w], BF16, tag=f"aw{rp}{kind}")
                nc.gpsimd.dma_start(out=t_c, in_=aw_v[:, off : off + w])
                off += w
                s_aw[(rp, kind)] = t_c
        s_rm = singles.tile([128, 4, 4], BF16, tag="rm")
        nc.gpsimd.dma_start(out=s_rm, in_=rm.rearrange("p (j c) -> p j c", j=4))

        # ---- W-lerp (bf16): rw0/1 = xsB[s] + c*(xsA[s]-xsB[s]);
        #      rw2/3 = xsA[s+2] + c*(xsB[s]-xsA[s+2])  (all even offsets) ----
        s_xsw = []
        for g, (g0, gn) in enumerate(ROW_GROUPS):
            t_w = singles.tile([100, gn, 4, 128], BF16, tag=f"xsw{g}")
            d0 = tmp_pool.tile([100, gn, 128], BF16, tag="d0")
            d1 = tmp_pool.tile([100, gn, 128], BF16, tag="d1")
            nc.vector.tensor_sub(d0, s_xa[g][:, :, 0:128], s_xb[g][:, :, 0:128])
            nc.vector.tensor_sub(d1, s_xb[g][:, :, 0:128], s_xa[g][:, :, 2:130])
            for rw, (coef, dt_, base) in enumerate(
                (
                    (0.375, 0, 0),
                    (0.125, 0, 0),
                    (0.875, 1, 2),
                    (0.625, 1, 2),
                )
            ):
                src_d = d0 if dt_ == 0 else d1
                src_x = (
                    s_xb[g][:, :, 0:128] if dt_ == 0 else s_xa[g][:, :, 2:130]
                )
                nc.vector.scalar_tensor_tensor(
                    out=t_w[:, :, rw, :],
                    in0=src_d,
                    scalar=coef,
                    in1=src_x,
                    op0=mult,
                    op1=add,
                )
            s_xsw.append(t_w)

        def xsw_row(l: int) -> bass.AP:
            for g, (g0, gn) in enumerate(ROW_GROUPS):
                if g0 <= l < g0 + gn:
                    return s_xsw[g][:, l - g0, :, :]
            raise IndexError(l)

        smat = fin.tile([64, 2, 512], F32, tag="smat")

        # ---- PE warmup: ~4.5us of back-to-back matmuls on the loaded
        # consts so the HAM clock gate releases (1.2 -> 2.4 GHz) before the
        # real work; results land in the paw tile and are overwritten ----
        # memset the paw banks once: the junk partition band 50:64 must hold
        # finite values (exp(0)=1) so zero-weight stat rows stay 0, not NaN
        pwarm = paw.tile([128, 4, 512], F32, tag="pw")
        nc.vector.memset(pwarm[:, :, :], 0.0)
        for i in range(80):
            nc.tensor.matmul(
                pwarm[0:8, 0, 0:50],
                s_aw[(0, "a0")][:, 0:8],
                s_aw[(0, "a0")][:, 0:50],
                start=True,
                stop=True,
                skip_group_check=True,
            )

        # ---- main loop ----
        from collections import deque

        pending = deque()

        def emit_stats(ps, taq, et, tau, t0, is_block_end):
            # the two rp accumulation chains target different 32-aligned
            # partition bases (= array column groups), so their matmuls can
            # stream through the PE concurrently; j-major emission keeps
            # adjacent instructions conflict-free
            for j in range(4):
                for rp in range(2):
                    base = 32 * (2 * tau + rp)
                    if j == 0:
                        rhs = taq[0:114, 2 * rp, :]
                        lhsT = s_rm[0:114, 0, :]
                    else:
                        rhs = et[0:111, rp, j - 1, :]
                        lhsT = s_rm[0:111, j, :]
                    nc.tensor.matmul(
                        ps[base : base + 4, :],
                        lhsT,
                        rhs,
                        start=(j == 0),
                        stop=(j == 3),
                        skip_group_check=True,
                        tile_position=(0, base),
                    )
            if is_block_end:
                sd = dpool.tile([128, 512], F32, tag="sd")
                nc.vector.tensor_copy(sd[0:100, :], ps[0:100, :])
                for t2 in range(2):
                    for rp in range(2):
                        j0 = 4 * (t0 + t2) + 2 * rp
                        b = 32 * (2 * t2 + rp)
                        eng = nc.sync if rp == 0 else nc.gpsimd
                        eng.dma_start(
                            out=smat[j0 : j0 + 2, :, :],
                            in_=sd[b : b + 4, :],
                        )

        ps = None
        for t in range(16):
            if t % 2 == 0:
                ps = pstat.tile([128, 512], F32, tag="ps")
            tau = t % 2
            pw = paw.tile([128, 4, 512], F32, tag="pw")
            taq = epool.tile([128, 4, 512], BF16, tag="taq")
            for rp in range(2):
                rhs = xsw_row(t + rp).rearrange("p q s -> p (q s)")
                # ui halves at column-group pairs {0,1} / {2,3} -> concurrent
                nc.tensor.matmul(
                    pw[0:50, 2 * rp, :], s_aw[(rp, "a0")], rhs,
                    start=True, stop=True, tile_position=(0, 0),
                )
                nc.tensor.matmul(
                    pw[64:114, 2 * rp, :], s_aw[(rp, "a1")], rhs,
                    start=True, stop=True, tile_position=(0, 64),
                )
                nc.tensor.matmul(
                    pw[0:47, 2 * rp + 1, :], s_aw[(rp, "w0")], rhs,
                    start=True, stop=True, tile_position=(0, 0),
                )
                nc.tensor.matmul(
                    pw[64:111, 2 * rp + 1, :], s_aw[(rp, "w1")], rhs,
                    start=True, stop=True, tile_position=(0, 64),
                )
            nc.scalar.activation(
                taq[0:114, :, :], pw[0:114, :, :], exp_fn, scale=-1.0
            )
            # reconstruction: E_j = E_{j-1} * q  (bf16 2x, both rp per op;
            # rows 47:64 are finite junk masked by zero stat weights)
            et = erp.tile([128, 2, 3, 512], BF16, tag="et")
            taq_v = taq.rearrange("p (rp aw) s -> p rp aw s", rp=2)
            nc.vector.tensor_mul(
                et[0:111, :, 0, :], taq_v[0:111, :, 0, :], taq_v[0:111, :, 1, :]
            )
            for j in (1, 2):
                nc.vector.tensor_mul(
                    et[0:111, :, j, :], et[0:111, :, j - 1, :], taq_v[0:111, :, 1, :]
                )
            pending.append((ps, taq, et, tau, t - tau, tau == 1))
            if len(pending) > 2:
                emit_stats(*pending.popleft())
        while pending:
            emit_stats(*pending.popleft())

        # ---- finalize: disp = S1 * recip(S0); un-permute (rw, s) -> w' ----
        rec = fin.tile([64, 512], F32, tag="rec")
        om = fin.tile([64, 512], F32, tag="om")
        nc.vector.reciprocal_approx_fast(out=rec, in_=smat[:, 0, :])
        nc.vector.tensor_mul(
            om.rearrange("j (s q) -> j q s", q=4),
            smat[:, 1, :].rearrange("j (q s) -> j q s", q=4),
            rec.rearrange("j (q s) -> j q s", q=4),
        )
        nc.sync.dma_start(out=outp[:, :], in_=om)

    nc.compile()
    return nc


_CACHE: dict = {}


def _shard_inputs(x: np.ndarray):
    """Edge-pad and slice per-core shards (layout + dtype cast only)."""
    xpad = np.pad(x[:, 0], ((0, 0), (1, 1), (1, 3), (1, 1)), mode="edge")
    aw, rm = _build_consts()
    bf = mybir.dt.np(BF16)
    in_maps = []
    for c in range(NCORES):
        b, q = divmod(c, 4)
        xs = xpad[b][:, 16 * q : 16 * q + 18, :]          # [50, 18, 130]
        xsd = np.concatenate([xs[:, 0:17, :], xs[:, 1:18, :]], axis=0)
        xsdA = np.ascontiguousarray(xsd.reshape(100, NROW * 130)).astype(bf)
        xsdB = np.zeros_like(xsd)
        xsdB[:, :, 0:129] = xsd[:, :, 1:130]
        xsdB = np.ascontiguousarray(xsdB.reshape(100, NROW * 130)).astype(bf)
        in_maps.append({"xsdA": xsdA, "xsdB": xsdB, "aw": aw, "rm": rm})
    return in_maps


def kernel(x: np.ndarray, _trace: bool = False, _tmpdir=None):
    x = np.asarray(x, dtype=np.float32)
    assert x.shape == (2, 1, 48, 64, 128), x.shape
    if "nc" not in _CACHE:
        _CACHE["nc"] = _build_nc()
    nc = _CACHE["nc"]
    in_maps = _shard_inputs(x)
    res = run_bass_kernel_spmd(
        nc, in_maps, list(range(NCORES)), trace=_trace, tmpdir=_tmpdir
    )
    out = np.zeros((2, 256, 512), dtype=np.float32)
    for c in range(NCORES):
        b, q = divmod(c, 4)
        out[b, 64 * q : 64 * (q + 1), :] = res.results[c]["out"]
    if _trace:
        return out, res
    return out


# revision 34
# speedup vs baseline: 1.2808x; 1.0544x over previous
"""Trainium2 Bass kernel for nn_Disp_61125974557155 (V2: segment-geometric exp).

Computes: trilinear upsample of a cost volume [B,1,48,64,128] ->
[B,193,256,512] (align_corners=False), softmin over disparity,
disparity regression -> [B,256,512].

Key idea: along d', the upsampled volume is piecewise-linear over 49
low-res segments (~4 d' each), so exp(-vol) is piecewise GEOMETRIC:
within segment s, e_j = A_s * q_s^j with A_s = exp(-(u_s + f0*Delta)),
q_s = exp(-c*Delta), c = 48/193. Instead of 193 exp rows per pixel the
scalar engine computes only ~194 rows per (t, r-pair) (A and q for two
H-phases), and the idle vector engine reconstructs the remaining rows
with chained bf16 multiplies (2x DVE mode). Edge segments (replicated
pads, Delta=0, 2 d's each) are folded into the j=0 stat weights; the
single 5-element segment gets its 5th row directly from the alpha
matmul (alpha + 4w column).

Per core (8 = 2 batches x 4 H'-quarters), t-major over 16 coarse rows:
  PE:  alpha/w matmul [100->100/94, 512] per (t, rp) (H-lerp folded),
       then 8 masked stat matmuls (M=4: S0/S1 x r-even/r-odd) per t,
       accumulated over layers j=0..3 in PSUM slots at partition 32*(t%4).
  ACT: one exp over [100, 4, 512] PSUM -> SBUF bf16 per t.
  DVE: W-lerp (bf16), 3 reconstruction TT muls per t, stat drains every
       4 t, partition-parallel finalize.
  DMA: stat scatter to pixel-major S0/S1, final divide, output.
"""

import numpy as np
from contextlib import ExitStack

import concourse.bass as bass
import concourse.bacc as bacc
import concourse.tile as tile
from concourse import mybir
from concourse.bass_utils import run_bass_kernel_spmd

F32 = mybir.dt.float32
BF16 = mybir.dt.bfloat16

MAXDISP = 192
DP = MAXDISP + 1
KD = 48
KP = KD + 2
NCORES = 8
C2 = KD / DP
WH = (0.625, 0.875, 0.125, 0.375)
NROW = 17
ROW_GROUPS = ((0, 1), (1, 1), (2, 2), (4, 4), (8, 4), (12, 4), (16, 1))


def _segments():
    segs = {}
    for dd in range(DP):
        i = (dd + 0.5) * C2 - 0.5
        fl = int(np.floor(i))
        segs.setdefault(fl, []).append((dd, i - fl))
    int_fls = [fl for fl in sorted(segs) if 0 <= fl <= 46]
    edge_fls = [-1, 47]
    five = [fl for fl in int_fls if len(segs[fl]) == 5]
    assert len(int_fls) == 47 and len(five) == 1
    return segs, int_fls, edge_fls, five[0]


def _build_consts():
    segs, int_fls, edge_fls, five_fl = _segments()

    def hrow(r, k, wt):
        v = np.zeros(100)
        v[k] = (1 - WH[r]) * wt
        v[50 + k] = WH[r] * wt
        return v

    # per (rp, r-parity ui): A-half [100, 50] = [47 interior | 2 edges |
    # 1 five-col], W-half [100, 47]. The two ui halves of each matmul run
    # at array column-group pairs {0,1} / {2,3} (partition bases 0 / 64).
    awA = np.zeros((2, 2, 100, 50))
    awW = np.zeros((2, 2, 100, 47))
    for rp in range(2):
        for ui, r in enumerate((2 * rp, 2 * rp + 1)):
            for s, fl in enumerate(int_fls):
                klo = fl + 1
                f0 = segs[fl][0][1]
                awA[rp][ui][:, s] = hrow(r, klo, 1 - f0) + hrow(r, klo + 1, f0)
                awW[rp][ui][:, s] = hrow(r, klo + 1, C2) + hrow(r, klo, -C2)
            for ei, fl in enumerate(edge_fls):
                klo = fl + 1
                f0 = segs[fl][0][1]
                awA[rp][ui][:, 47 + ei] = (
                    hrow(r, klo, 1 - f0) + hrow(r, klo + 1, f0)
                )
            klo = five_fl + 1
            f0 = segs[five_fl][0][1]
            a_col = hrow(r, klo, 1 - f0) + hrow(r, klo + 1, f0)
            w_col = hrow(r, klo + 1, C2) + hrow(r, klo, -C2)
            awA[rp][ui][:, 49] = a_col + 4 * w_col

    # stat cols ordered (S0_r0, S1_r0, S0_r1, S1_r1): one DMA per (tau, rp)
    # scatters straight into the interleaved [j, (S0|S1), 512] S-matrix.
    # Rows follow the split taq layout: ui-block at partition base 64*ui =
    # [47 interior | 2 edges | 1 five | 14 junk(zero-weight)].
    rmA = np.zeros((128, 4))
    rmE = np.zeros((3, 128, 4))
    for ui in range(2):
        base = 64 * ui
        for s, fl in enumerate(int_fls):
            ds = [t[0] for t in segs[fl]]
            rmA[base + s, 2 * ui] = 1
            rmA[base + s, 2 * ui + 1] = ds[0]
            for j in (1, 2, 3):
                rmE[j - 1][base + s, 2 * ui] = 1
                rmE[j - 1][base + s, 2 * ui + 1] = ds[j]
        for ei, fl in enumerate(edge_fls):
            ds = [t[0] for t in segs[fl]]
            rmA[base + 47 + ei, 2 * ui] = 2
            rmA[base + 47 + ei, 2 * ui + 1] = ds[0] + ds[1]
        ds = [t[0] for t in segs[five_fl]]
        rmA[base + 49, 2 * ui] = 1
        rmA[base + 49, 2 * ui + 1] = ds[4]

    bf = mybir.dt.np(BF16)
    aw = np.concatenate(
        [awA[0][0], awA[0][1], awW[0][0], awW[0][1],
         awA[1][0], awA[1][1], awW[1][0], awW[1][1]], axis=1
    )  # [100, 388]
    rm = np.concatenate([rmA] + [rmE[j] for j in range(3)], axis=1)  # [128, 16]
    return aw.astype(bf), rm.astype(bf)


def _build_nc() -> bass.Bass:
    nc = bacc.Bacc()
    # xsdA/xsdB: host-duplicated, B shifted by one w-col, so every lerp
    # slice starts at an even element offset (keeps DVE 2x eligibility).
    xsdA = nc.declare_dram_parameter("xsdA", [100, NROW * 130], BF16, isOutput=False)
    xsdB = nc.declare_dram_parameter("xsdB", [100, NROW * 130], BF16, isOutput=False)
    aw = nc.declare_dram_parameter("aw", [100, 388], BF16, isOutput=False)
    rm = nc.declare_dram_parameter("rm", [128, 16], BF16, isOutput=False)
    outp = nc.declare_dram_parameter("out", [64, 512], F32, isOutput=True)

    xa_v = xsdA.rearrange("p (h w) -> p h w", h=NROW)
    xb_v = xsdB.rearrange("p (h w) -> p h w", h=NROW)
    aw_v = aw  # cols: rp0A(100) rp0W(94) rp1A(100) rp1W(94)

    mult = mybir.AluOpType.mult
    add = mybir.AluOpType.add
    exp_fn = mybir.ActivationFunctionType.Exp

    with ExitStack() as ctx:
        tc = ctx.enter_context(tile.TileContext(nc))
        singles = ctx.enter_context(tc.tile_pool(name="singles", bufs=1))
        tmp_pool = ctx.enter_context(tc.tile_pool(name="tmp", bufs=2))
        epool = ctx.enter_context(tc.tile_pool(name="epool", bufs=4))
        erp = ctx.enter_context(tc.tile_pool(name="erp", bufs=3))
        dpool = ctx.enter_context(tc.tile_pool(name="dpool", bufs=2))
        fin = ctx.enter_context(tc.tile_pool(name="fin", bufs=1))
        paw = ctx.enter_context(tc.tile_pool(name="paw", bufs=1, space="PSUM"))
        pstat = ctx.enter_context(tc.tile_pool(name="pstat", bufs=2, space="PSUM"))

        # ---- loads ----
        s_xa, s_xb = [], []
        for g, (g0, gn) in enumerate(ROW_GROUPS):
            t_xa = singles.tile([100, gn, 130], BF16, tag=f"xa{g}")
            nc.sync.dma_start(out=t_xa, in_=xa_v[:, g0 : g0 + gn, :])
            t_xb = singles.tile([100, gn, 130], BF16, tag=f"xb{g}")
            nc.scalar.dma_start(out=t_xb, in_=xb_v[:, g0 : g0 + gn, :])
            s_xa.append(t_xa)
            s_xb.append(t_xb)
        s_aw = {}
        off = 0
        for rp in range(2):
            for kind, w in (("a0", 50), ("a1", 50), ("w0", 47), ("w1", 47)):
                t_c = singles.tile([100, w], BF16, tag=f"aw{rp}{kind}")
                nc.gpsimd.dma_start(out=t_c, in_=aw_v[:, off : off + w])
                off += w
                s_aw[(rp, kind)] = t_c
        s_rm = singles.tile([128, 4, 4], BF16, tag="rm")
        nc.gpsimd.dma_start(out=s_rm, in_=rm.rearrange("p (j c) -> p j c", j=4))

        # ---- W-lerp (bf16): rw0/1 = xsB[s] + c*(xsA[s]-xsB[s]);
        #      rw2/3 = xsA[s+2] + c*(xsB[s]-xsA[s+2])  (all even offsets) ----
        s_xsw = []
        for g, (g0, gn) in enumerate(ROW_GROUPS):
            t_w = singles.tile([100, gn, 4, 128], BF16, tag=f"xsw{g}")
            d0 = tmp_pool.tile([100, gn, 128], BF16, tag="d0")
            d1 = tmp_pool.tile([100, gn, 128], BF16, tag="d1")
            nc.vector.tensor_sub(d0, s_xa[g][:, :, 0:128], s_xb[g][:, :, 0:128])
            nc.vector.tensor_sub(d1, s_xb[g][:, :, 0:128], s_xa[g][:, :, 2:130])
            for rw, (coef, dt_, base) in enumerate(
                (
                    (0.375, 0, 0),
                    (0.125, 0, 0),
                    (0.875, 1, 2),
                    (0.625, 1, 2),
                )
            ):
                src_d = d0 if dt_ == 0 else d1
                src_x = (
                    s_xb[g][:, :, 0:128] if dt_ == 0 else s_xa[g][:, :, 2:130]
                )
                # tensor_scalar (4x mode) + tensor_tensor add (2x) beats the
                # single scalar_tensor_tensor, which only has a 1x uop
                cd = tmp_pool.tile([100, gn, 128], BF16, tag="cd")
                nc.vector.tensor_scalar_mul(cd, src_d, coef)
                nc.vector.tensor_add(t_w[:, :, rw, :], cd, src_x)
            s_xsw.append(t_w)

        def xsw_row(l: int) -> bass.AP:
            for g, (g0, gn) in enumerate(ROW_GROUPS):
                if g0 <= l < g0 + gn:
                    return s_xsw[g][:, l - g0, :, :]
            raise IndexError(l)

        smat = fin.tile([64, 2, 512], F32, tag="smat")

        # ---- PE warmup: ~4.5us of back-to-back matmuls on the loaded
        # consts so the HAM clock gate releases (1.2 -> 2.4 GHz) before the
        # real work; results land in the paw tile and are overwritten ----
        # memset the paw banks once: the junk partition band 50:64 must hold
        # finite values (exp(0)=1) so zero-weight stat rows stay 0, not NaN
        pwarm = paw.tile([128, 4, 512], F32, tag="pw")
        nc.vector.memset(pwarm[:, :, :], 0.0)
        for i in range(80):
            nc.tensor.matmul(
                pwarm[0:8, 0, 0:50],
                s_aw[(0, "a0")][:, 0:8],
                s_aw[(0, "a0")][:, 0:50],
                start=True,
                stop=True,
                skip_group_check=True,
            )

        # ---- main loop ----
        from collections import deque

        pending = deque()

        def emit_stats(ps, taq, et, tau, t0, is_block_end):
            # the two rp accumulation chains target different 32-aligned
            # partition bases (= array column groups), so their matmuls can
            # stream through the PE concurrently; j-major emission keeps
            # adjacent instructions conflict-free
            for j in range(4):
                for rp in range(2):
                    base = 32 * (2 * tau + rp)
                    if j == 0:
                        rhs = taq[0:114, 2 * rp, :]
                        lhsT = s_rm[0:114, 0, :]
                    else:
                        rhs = et[0:111, rp, j - 1, :]
                        lhsT = s_rm[0:111, j, :]
                    nc.tensor.matmul(
                        ps[base : base + 4, :],
                        lhsT,
                        rhs,
                        start=(j == 0),
                        stop=(j == 3),
                        skip_group_check=True,
                        tile_position=(0, base),
                    )
            if is_block_end:
                sd = dpool.tile([128, 512], F32, tag="sd")
                nc.vector.tensor_copy(sd[0:100, :], ps[0:100, :])
                for t2 in range(2):
                    for rp in range(2):
                        j0 = 4 * (t0 + t2) + 2 * rp
                        b = 32 * (2 * t2 + rp)
                        eng = nc.sync if rp == 0 else nc.gpsimd
                        eng.dma_start(
                            out=smat[j0 : j0 + 2, :, :],
                            in_=sd[b : b + 4, :],
                        )

        ps = None
        for t in range(16):
            if t % 2 == 0:
                ps = pstat.tile([128, 512], F32, tag="ps")
            tau = t % 2
            pw = paw.tile([128, 4, 512], F32, tag="pw")
            taq = epool.tile([128, 4, 512], BF16, tag="taq")
            for rp in range(2):
                rhs = xsw_row(t + rp).rearrange("p q s -> p (q s)")
                # ui halves at column-group pairs {0,1} / {2,3} -> concurrent
                nc.tensor.matmul(
                    pw[0:50, 2 * rp, :], s_aw[(rp, "a0")], rhs,
                    start=True, stop=True, tile_position=(0, 0),
                )
                nc.tensor.matmul(
                    pw[64:114, 2 * rp, :], s_aw[(rp, "a1")], rhs,
                    start=True, stop=True, tile_position=(0, 64),
                )
                nc.tensor.matmul(
                    pw[0:47, 2 * rp + 1, :], s_aw[(rp, "w0")], rhs,
                    start=True, stop=True, tile_position=(0, 0),
                )
                nc.tensor.matmul(
                    pw[64:111, 2 * rp + 1, :], s_aw[(rp, "w1")], rhs,
                    start=True, stop=True, tile_position=(0, 64),
                )
            nc.scalar.activation(
                taq[0:114, :, :], pw[0:114, :, :], exp_fn, scale=-1.0
            )
            # reconstruction: E_j = E_{j-1} * q  (bf16 2x, both rp per op;
            # rows 47:64 are finite junk masked by zero stat weights)
            et = erp.tile([128, 2, 3, 512], BF16, tag="et")
            taq_v = taq.rearrange("p (rp aw) s -> p rp aw s", rp=2)
            nc.vector.tensor_mul(
                et[0:111, :, 0, :], taq_v[0:111, :, 0, :], taq_v[0:111, :, 1, :]
            )
            for j in (1, 2):
                nc.vector.tensor_mul(
                    et[0:111, :, j, :], et[0:111, :, j - 1, :], taq_v[0:111, :, 1, :]
                )
            pending.append((ps, taq, et, tau, t - tau, tau == 1))
            if len(pending) > 2:
                emit_stats(*pending.popleft())
        while pending:
            emit_stats(*pending.popleft())

        # ---- finalize: disp = S1 * recip(S0); un-permute (rw, s) -> w' ----
        rec = fin.tile([64, 512], F32, tag="rec")
        om = fin.tile([64, 512], F32, tag="om")
        nc.vector.reciprocal_approx_fast(out=rec, in_=smat[:, 0, :])
        nc.vector.tensor_mul(
            om.rearrange("j (s q) -> j q s", q=4),
            smat[:, 1, :].rearrange("j (q s) -> j q s", q=4),
            rec.rearrange("j (q s) -> j q s", q=4),
        )
        nc.sync.dma_start(out=outp[:, :], in_=om)

    nc.compile()
    return nc


_CACHE: dict = {}


def _shard_inputs(x: np.ndarray):
    """Edge-pad and slice per-core shards (layout + dtype cast only)."""
    xpad = np.pad(x[:, 0], ((0, 0), (1, 1), (1, 3), (1, 1)), mode="edge")
    aw, rm = _build_consts()
    bf = mybir.dt.np(BF16)
    in_maps = []
    for c in range(NCORES):
        b, q = divmod(c, 4)
        xs = xpad[b][:, 16 * q : 16 * q + 18, :]          # [50, 18, 130]
        xsd = np.concatenate([xs[:, 0:17, :], xs[:, 1:18, :]], axis=0)
        xsdA = np.ascontiguousarray(xsd.reshape(100, NROW * 130)).astype(bf)
        xsdB = np.zeros_like(xsd)
        xsdB[:, :, 0:129] = xsd[:, :, 1:130]
        xsdB = np.ascontiguousarray(xsdB.reshape(100, NROW * 130)).astype(bf)
        in_maps.append({"xsdA": xsdA, "xsdB": xsdB, "aw": aw, "rm": rm})
    return in_maps


def kernel(x: np.ndarray, _trace: bool = False, _tmpdir=None):
    x = np.asarray(x, dtype=np.float32)
    assert x.shape == (2, 1, 48, 64, 128), x.shape
    if "nc" not in _CACHE:
        _CACHE["nc"] = _build_nc()
    nc = _CACHE["nc"]
    in_maps = _shard_inputs(x)
    res = run_bass_kernel_spmd(
        nc, in_maps, list(range(NCORES)), trace=_trace, tmpdir=_tmpdir
    )
    out = np.zeros((2, 256, 512), dtype=np.float32)
    for c in range(NCORES):
        b, q = divmod(c, 4)
        out[b, 64 * q : 64 * (q + 1), :] = res.results[c]["out"]
    if _trace:
        return out, res
    return out
